# revision 64
# baseline (speedup 1.0000x reference)
"""DigitCaps dynamic-routing kernel for 8 TRN2 NeuronCores.

Problem: x [512, 1152, 8], W [1152, 10, 16, 8] -> v [512, 10, 16]
  u_hat[b,i,o,p] = sum_q W[i,o,p,q] x[b,i,q]
  3 routing iterations; b_ij update uses batch-mean agreement (global).

Strategy (pure data parallel over batch, 64 rows/core):
  Never materialize u_hat. Per iteration:
    s[b,op]  = sum_{i,q} (c[i,o] W[i,o,p,q]) x[b,i,q]      (PE, contract 9216)
    v        = squash(s)                                    (DVE/ACT)
    C[iq,op] = sum_b x[b,iq] v[b,op]                        (PE, contract 64)
    u[i,o]   = sum_{p,q} W_t[iq,op] * C[iq,op]              (ACT copy + DVE)
    AllReduce(u) across 8 cores; b += u/512.
  Iteration 1 uses uniform c = 1/1152 (softmax of zeros) so s comes straight
  from W with a folded 1/1152 scale; iteration 3 skips the dead b-update.

  Matmul inputs (x, W, A, v) are stored fp16: TRN2 PE runs fp32 matmuls at
  4 cycles/row (two split passes) but fp16 at 1; all accumulation stays fp32
  and every operand here is unit-scale, so fp16 keeps ~3e-4 relative error.
  W is uploaded host-transposed to [i, (q, o, p)] so every on-chip consumer
  (s-matmul rhs, A-build, agreement elementwise) reads it with a contiguous
  free dim — required for the DVE 16-bit 2x perf mode.
"""

import os
import sys

import numpy as np

for _p in ("/opt/trn_rl_repo",):
    if _p not in sys.path:
        sys.path.insert(0, _p)

import concourse.bass as bass
import concourse.tile as tile
from concourse import bacc, mybir
from concourse.bass import ts
from concourse.bass_utils import run_bass_kernel_spmd
from concourse.masks import make_identity

B = 512
I, Q = 1152, 8
O, P = 10, 16
OP = O * P          # 160
IQ = I * Q          # 9216
NCORES = 8
BL = B // NCORES    # 64
NT = I // 128       # 9 i-tiles
NUM_ITERS = 3
F32 = mybir.dt.float32
F16 = mybir.dt.float16
NP_IN = np.float16


def _x_block(xw, t, q):
    """x[b, i*8+q] for i in [128t, 128t+128) as a [64, 128] stride-8 AP.

    xw is [128, 5120]: rows 0:64 hold x cols [0, 4096) (i-tiles 0..3),
    rows 64:128 hold x cols [4096, 9216) (i-tiles 4..8).
    """
    if t < 4:
        base = xw[0:64, 0:4096].rearrange("b (i q) -> b i q", q=Q)
        return base[:, ts(t, 128), q]
    base = xw[64:128, 0:5120].rearrange("b (i q) -> b i q", q=Q)
    return base[:, ts(t - 4, 128), q]


def _w_qop(w_sb, t):
    """W tile t viewed as [128(i), 8(q), 10(o), 16(p)] — contiguous."""
    return w_sb[:, t, :].rearrange("p (q o pp) -> p q o pp", o=O, pp=P, q=Q)


def build_digitcaps(dt_in=F16):
    nc = bacc.Bacc(
        "TRN2", target_bir_lowering=False, debug=False, num_devices=NCORES
    )
    x_d = nc.dram_tensor("x", [BL, IQ], dt_in, kind="ExternalInput")
    w_d = nc.dram_tensor("w", [I, Q * O * P], dt_in, kind="ExternalInput")
    v_d = nc.dram_tensor("v", [BL, OP], F32, kind="ExternalOutput")

    with tile.TileContext(nc) as tc:
        with (
            tc.tile_pool(name="persist", bufs=1) as persist,
            tc.tile_pool(name="scratch", bufs=2) as scratch,
            tc.tile_pool(name="pp_small", bufs=2, space="PSUM") as pp_small,
            tc.tile_pool(name="pp_s", bufs=1, space="PSUM") as pp_s,
            tc.tile_pool(name="pp_c", bufs=2, space="PSUM") as pp_c,
            tc.tile_pool(name="dram", bufs=1, space="DRAM") as dram,
        ):
            # Dummy collective issued as the very first thing: the first
            # collective in a NEFF absorbs the cross-core launch skew
            # (measured ~16us), so pay it here, overlapped with the x/W DMAs
            # and transposes, instead of inside the first real AllReduce.
            dcc_s = persist.tile([1, 8], F32, tag="dcc_s")
            nc.vector.memset(dcc_s, 0.0)
            dcc_i = dram.tile([1, 8], F32, tag="dcc_i")
            dcc_o = dram.tile([1, 8], F32, tag="dcc_o")
            nc.gpsimd.dma_start(out=dcc_i[:, :], in_=dcc_s[:, :])
            nc.gpsimd.collective_compute(
                "AllReduce",
                mybir.AluOpType.add,
                replica_groups=[list(range(NCORES))],
                ins=[dcc_i[:, :]],
                outs=[dcc_o[:, :]],
            )

            # ---- constants ----
            # id64 twice: transposes of x blocks living at partition base 0
            # (i-tiles 0..3) and base 64 (i-tiles 4..8) need a same-base rhs.
            id64 = persist.tile([128, 64], dt_in, tag="id64")
            make_identity(nc, id64[0:64, :])
            nc.sync.dma_start(out=id64[64:128, :], in_=id64[0:64, :])
            id128 = persist.tile([128, 128], F32, tag="id128")
            make_identity(nc, id128)
            eps_c = persist.tile([64, 1], F32, tag="eps_c")
            nc.vector.memset(eps_c, 1e-8)
            ones1 = persist.tile([1, 64], F32, tag="ones1")
            nc.vector.memset(ones1, 1.0)

            # ---- load x (four concurrent DMAs, two per partition half) ----
            xw = persist.tile([128, 5120], dt_in, tag="xw")
            nc.sync.dma_start(out=xw[0:64, 0:2048], in_=x_d[:, 0:2048])
            nc.sync.dma_start(out=xw[0:64, 2048:4096], in_=x_d[:, 2048:4096])
            nc.sync.dma_start(out=xw[64:128, 0:2560], in_=x_d[:, 4096:6656])
            nc.sync.dma_start(out=xw[64:128, 2560:5120], in_=x_d[:, 6656:9216])

            # ---- load W (9 tiles; per-tile DMA for queue parallelism) ----
            w_sb = persist.tile([128, NT, Q * O * P], dt_in, tag="w_sb")
            w_rows = w_d.rearrange("(t p) f -> t p f", p=128)
            for t in range(NT):
                nc.sync.dma_start(out=w_sb[:, t, :], in_=w_rows[t])



            # ---- transpose x into xT[(t,q) blocks of [128(i), 64(b)]] ----
            xT = persist.tile([128, NT * Q * 64], dt_in, tag="xT")
            for t in range(NT):
                for q in range(Q):
                    pt = pp_small.tile([128, 64], dt_in, tag="pt", name="pt")
                    ident = id64[0:64, :] if t < 4 else id64[64:128, :]
                    nc.tensor.transpose(pt, _x_block(xw, t, q), ident)
                    # split PSUM->SBUF copies between ACT and DVE
                    if (t * Q + q) % 2 == 0:
                        nc.scalar.copy(xT[:, ts(t * Q + q, 64)], pt)
                    else:
                        nc.vector.tensor_copy(xT[:, ts(t * Q + q, 64)], pt)

            # ---- persistent routing state ----
            bT = persist.tile([10, I], F32, tag="bT")          # logits, [o, i]
            u_acc = persist.tile([128, NT, O], F32, tag="u_acc")
            A = persist.tile([128, NT, Q, O, P], dt_in, tag="A")

            # fp16 collective payload: ~9us of the ~13-18us AllReduce cost
            # scales with size, and fp16 logits cost only ~3e-4 relative.
            cc_in = dram.tile([10, I], F16, tag="cc_in")
            cc_out = dram.tile([10, I], F16, tag="cc_out")

            for k in range(NUM_ITERS):
                # ---- c = softmax(b) over i (skip: iter 0 is uniform) ----
                if k > 0:
                    negmax = scratch.tile([10, 1], F32, tag="negmax", name="negmax")
                    nc.vector.tensor_reduce(
                        negmax, bT, axis=mybir.AxisListType.X,
                        op=mybir.AluOpType.max, negate=True,
                    )
                    eT = scratch.tile([10, I], F32, tag="eT", name="eT")
                    esum = scratch.tile([10, 1], F32, tag="esum", name="esum")
                    nc.scalar.activation(
                        out=eT, in_=bT, func=mybir.ActivationFunctionType.Exp,
                        bias=negmax, scale=1.0, accum_out=esum,
                    )
                    nc.vector.reciprocal(esum, esum)
                    cT = scratch.tile([10, I], F32, tag="cT", name="cT")
                    nc.vector.tensor_scalar_mul(cT, eT, esum)
                    # per i-tile: transpose c back to [i, o] (fp16), then
                    # A = c * W broadcast-mult split across DVE and GpSimd.
                    for t in range(NT):
                        pc = pp_small.tile([128, 10], F32, tag="pt", name="pc")
                        nc.tensor.transpose(
                            pc, cT[:, ts(t, 128)], id128[0:10, 0:10]
                        )
                        c16n = scratch.tile(
                            [128, O], dt_in, tag="c16n", name="c16n", bufs=3
                        )
                        nc.scalar.copy(c16n, pc)
                        # split the broadcast mult across DVE and GpSimd so
                        # neither engine gates the s-matmuls behind it; the
                        # slower GpSimd (2.9us/tile vs DVE 1.4us) takes only
                        # the last three tiles.
                        eng = nc.vector if t < 6 else nc.gpsimd
                        eng.tensor_mul(
                            A[:, t],
                            _w_qop(w_sb, t),
                            c16n[:, None, :, None].broadcast_to(
                                [128, Q, O, P]
                            ),
                        )

                # ---- s = sum_{i,q} A[iq, op] x[b, iq]  (72 matmuls, one PSUM) ----
                ps = pp_s.tile([64, OP], F32, tag="ps", name="ps")
                first, last = (0, 0), (NT - 1, Q - 1)
                for t in range(NT):
                    for q in range(Q):
                        rhs = (
                            _w_qop(w_sb, t)[:, q] if k == 0 else A[:, t, q]
                        )
                        nc.tensor.matmul(
                            ps,
                            lhsT=xT[:, ts(t * Q + q, 64)],
                            rhs=rhs,
                            start=(t, q) == first,
                            stop=(t, q) == last,
                        )
                s_sb = scratch.tile([64, O, P], F32, tag="s_sb", name="s_sb")
                nc.scalar.mul(
                    s_sb.rearrange("b o p -> b (o p)"), ps,
                    1.0 / I if k == 0 else 1.0,
                )

                # ---- v = squash(s) ----
                sqs = scratch.tile([64, O, P], F32, tag="sqs", name="sqs")
                nc.vector.tensor_mul(sqs, s_sb, s_sb)
                sq = scratch.tile([64, O], F32, tag="sq", name="sq")
                nc.vector.reduce_sum(sq, sqs, axis=mybir.AxisListType.X)
                t1 = scratch.tile([64, O], F32, tag="t1", name="t1")
                nc.scalar.activation(
                    out=t1, in_=sq, func=mybir.ActivationFunctionType.Sqrt,
                    bias=eps_c, scale=1.0,
                )
                t2 = scratch.tile([64, O], F32, tag="t2", name="t2")
                # t2 = (sq + 1) * sqrt(sq + eps), fused in one DVE op
                nc.vector.scalar_tensor_tensor(
                    out=t2, in0=sq, scalar=1.0, in1=t1,
                    op0=mybir.AluOpType.add, op1=mybir.AluOpType.mult,
                )
                nc.vector.reciprocal(t2, t2)
                nc.vector.tensor_mul(t2, sq, t2)   # t2 = sq/((1+sq)*sqrt(sq+eps))
                v_sb = scratch.tile([64, O, P], F32, tag="v_sb", name="v_sb")
                nc.vector.tensor_mul(
                    v_sb, s_sb, t2[:, :, None].broadcast_to([64, O, P])
                )

                if k == NUM_ITERS - 1:
                    nc.sync.dma_start(
                        out=v_d[:, :],
                        in_=v_sb.rearrange("b o p -> b (o p)"),
                    )
                    continue

                # fp16 copies of v (partition bases 0 and 64) for the
                # agreement matmul rhs, matching each x block's base.
                v16 = scratch.tile([128, OP], dt_in, tag="v16", name="v16")
                vf = v_sb.rearrange("b o p -> b (o p)")
                nc.scalar.copy(v16[0:64, :], vf)
                nc.scalar.copy(v16[64:128, :], vf)

                # ---- agreement: u[i,o] = sum_{b,p,q} W x v ----
                # C lands in PSUM fp32; ACT copies it to fp16 SBUF so the
                # DVE mult runs in the 16-bit 2x perf mode. The q-reduction
                # is a 2x fp16 add-tree; only the final p-reduction runs at
                # the DVE's 1x reduce rate.
                uT = scratch.tile([10, I], dt_in, tag="uT", name="uT")
                for t in range(NT):
                    prods = []
                    for h in range(2):       # q in quads
                        pc4 = pp_c.tile([128, 1024], F32, tag="pc4", name="pc4")
                        for qq in range(4):
                            q = h * 4 + qq
                            nc.tensor.matmul(
                                pc4[:, qq * 256 : qq * 256 + OP],
                                lhsT=_x_block(xw, t, q),
                                rhs=v16[0:64] if t < 4 else v16[64:128],
                                start=True,
                                stop=True,
                            )
                        cview = pc4.rearrange("p (q f) -> p q f", q=4)[
                            :, :, 0:OP
                        ].rearrange("p q (o pp) -> p q o pp", o=O)
                        prod = scratch.tile(
                            [128, 4, O, P], dt_in, tag=f"prod{h}",
                            name="prod", bufs=2,
                        )
                        if t % 2 == 0:
                            # DVE mult straight from PSUM (1x rate) —
                            # offloads the near-saturated ACT engine
                            nc.vector.tensor_mul(
                                prod, cview, _w_qop(w_sb, t)[:, ts(h, 4)]
                            )
                        else:
                            # ACT copies C to fp16 SBUF so the DVE mult
                            # runs in the 16-bit 2x perf mode
                            c16 = scratch.tile(
                                [128, 4, O, P], dt_in, tag="c16",
                                name="c16", bufs=3,
                            )
                            nc.scalar.copy(c16, cview)
                            nc.vector.tensor_mul(
                                prod, c16, _w_qop(w_sb, t)[:, ts(h, 4)]
                            )
                        prods.append(prod)
                    s4 = scratch.tile(
                        [128, 4, O, P], dt_in, tag="s4", name="s4"
                    )
                    nc.vector.tensor_add(s4, prods[0], prods[1])
                    s2 = scratch.tile(
                        [128, 2, O, P], dt_in, tag="s2", name="s2"
                    )
                    nc.vector.tensor_add(s2, s4[:, 0:2], s4[:, 2:4])
                    s1 = scratch.tile([128, O, P], dt_in, tag="s1", name="s1")
                    nc.vector.tensor_add(s1, s2[:, 0], s2[:, 1])
                    nc.vector.tensor_reduce(
                        u_acc[:, t, :], s1, axis=mybir.AxisListType.X,
                        op=mybir.AluOpType.add,
                    )
                # ---- transpose u -> [o, i], AllReduce, update b ----
                for t in range(NT):
                    pu = pp_small.tile([10, 128], F32, tag="pt", name="pu")
                    nc.tensor.transpose(pu, u_acc[:, t, :], id128)
                    nc.scalar.copy(uT[:, ts(t, 128)], pu)
                nc.gpsimd.dma_start(out=cc_in[:], in_=uT[:])
                nc.gpsimd.collective_compute(
                    "AllReduce",
                    mybir.AluOpType.add,
                    replica_groups=[list(range(NCORES))],
                    ins=[cc_in[:]],
                    outs=[cc_out[:]],
                )
                # Filler matmuls: the AllReduce stalls the PE ~12-20us and
                # the HAM clock-gate then holds the PE at reduced speed for
                # the next phase. Keep it busy with throwaway matmuls during
                # the collective wait; reading uT makes them start only once
                # the collective inputs are ready (not earlier).
                # the first AllReduce runs ~19us, the second ~12us
                for f in range(60 if k == 0 else 36):
                    pf = pp_s.tile([64, OP], F32, tag="pf", name="pf")
                    nc.tensor.matmul(
                        pf,
                        lhsT=id64[0:10, :],
                        rhs=uT[:, ts(f % 7, OP)],
                        start=True,
                        stop=True,
                    )
                uTr = scratch.tile([10, I], dt_in, tag="uTr", name="uTr")
                nc.gpsimd.dma_start(out=uTr[:], in_=cc_out[:])
                if k == 0:
                    nc.vector.tensor_scalar_mul(bT, uTr, 1.0 / B)
                else:
                    nc.vector.scalar_tensor_tensor(
                        out=bT, in0=uTr, scalar=1.0 / B, in1=bT,
                        op0=mybir.AluOpType.mult, op1=mybir.AluOpType.add,
                    )

    nc.compile()
    return nc


_CACHE = {}


def _get_nc():
    if "nc" not in _CACHE:
        _CACHE["nc"] = build_digitcaps()
    return _CACHE["nc"]


def _prep_w(W: np.ndarray) -> np.ndarray:
    # [i, o, p, q] -> [i, (q, o, p)] so on-chip reads are contiguous
    return np.ascontiguousarray(
        W.transpose(0, 3, 1, 2).astype(NP_IN).reshape(I, Q * O * P)
    )


def kernel(x: np.ndarray, W: np.ndarray) -> np.ndarray:
    assert x.shape == (B, I, Q) and W.shape == (I, O, P, Q)
    nc = _get_nc()
    xf = np.ascontiguousarray(x.reshape(B, IQ).astype(NP_IN))
    wf = _prep_w(W)
    in_maps = [
        {"x": xf[c * BL : (c + 1) * BL], "w": wf} for c in range(NCORES)
    ]
    res = run_bass_kernel_spmd(nc, in_maps, list(range(NCORES)))
    out = np.concatenate([res.results[c]["v"] for c in range(NCORES)], axis=0)
    return out.reshape(B, O, P)


# revision 65
# speedup vs baseline: 1.0648x; 1.0648x over previous
"""DigitCaps dynamic-routing kernel for 8 TRN2 NeuronCores.

Problem: x [512, 1152, 8], W [1152, 10, 16, 8] -> v [512, 10, 16]
  u_hat[b,i,o,p] = sum_q W[i,o,p,q] x[b,i,q]
  3 routing iterations; b_ij update uses batch-mean agreement (global).

Strategy (pure data parallel over batch, 64 rows/core):
  Never materialize u_hat. Per iteration:
    s[b,op]  = sum_{i,q} (c[i,o] W[i,o,p,q]) x[b,i,q]      (PE, contract 9216)
    v        = squash(s)                                    (DVE/ACT)
    C[iq,op] = sum_b x[b,iq] v[b,op]                        (PE, contract 64)
    u[i,o]   = sum_{p,q} W_t[iq,op] * C[iq,op]              (ACT copy + DVE)
    AllReduce(u) across 8 cores; b += u/512.
  Iteration 1 uses uniform c = 1/1152 (softmax of zeros) so s comes straight
  from W with a folded 1/1152 scale; iteration 3 skips the dead b-update.

  Matmul inputs (x, W, A, v) are stored fp16: TRN2 PE runs fp32 matmuls at
  4 cycles/row (two split passes) but fp16 at 1; all accumulation stays fp32
  and every operand here is unit-scale, so fp16 keeps ~3e-4 relative error.
  W is uploaded host-transposed to [i, (q, o, p)] so every on-chip consumer
  (s-matmul rhs, A-build, agreement elementwise) reads it with a contiguous
  free dim — required for the DVE 16-bit 2x perf mode.
"""

import os
import sys

import numpy as np

for _p in ("/opt/trn_rl_repo",):
    if _p not in sys.path:
        sys.path.insert(0, _p)

import concourse.bass as bass
import concourse.tile as tile
from concourse import bacc, mybir
from concourse.bass import ts
from concourse.bass_utils import run_bass_kernel_spmd
from concourse.masks import make_identity

B = 512
I, Q = 1152, 8
O, P = 10, 16
OP = O * P          # 160
IQ = I * Q          # 9216
NCORES = 8
BL = B // NCORES    # 64
NT = I // 128       # 9 i-tiles
NUM_ITERS = 3
F32 = mybir.dt.float32
F16 = mybir.dt.float16
NP_IN = np.float16


def _x_block(xw, t, q):
    """x[b, i*8+q] for i in [128t, 128t+128) as a [64, 128] stride-8 AP.

    xw is [128, 5120]: rows 0:64 hold x cols [0, 4096) (i-tiles 0..3),
    rows 64:128 hold x cols [4096, 9216) (i-tiles 4..8).
    """
    if t < 4:
        base = xw[0:64, 0:4096].rearrange("b (i q) -> b i q", q=Q)
        return base[:, ts(t, 128), q]
    base = xw[64:128, 0:5120].rearrange("b (i q) -> b i q", q=Q)
    return base[:, ts(t - 4, 128), q]


def _w_qop(w_sb, t):
    """W tile t viewed as [128(i), 8(q), 10(o), 16(p)] — contiguous."""
    return w_sb[:, t, :].rearrange("p (q o pp) -> p q o pp", o=O, pp=P, q=Q)


def build_digitcaps(dt_in=F16):
    nc = bacc.Bacc(
        "TRN2", target_bir_lowering=False, debug=False, num_devices=NCORES
    )
    x_d = nc.dram_tensor("x", [BL, IQ], dt_in, kind="ExternalInput")
    w_d = nc.dram_tensor("w", [I, Q * O * P], dt_in, kind="ExternalInput")
    v_d = nc.dram_tensor("v", [BL, OP], F32, kind="ExternalOutput")

    with tile.TileContext(nc) as tc:
        with (
            tc.tile_pool(name="persist", bufs=1) as persist,
            tc.tile_pool(name="scratch", bufs=2) as scratch,
            tc.tile_pool(name="pp_small", bufs=2, space="PSUM") as pp_small,
            tc.tile_pool(name="pp_s", bufs=1, space="PSUM") as pp_s,
            tc.tile_pool(name="pp_c", bufs=2, space="PSUM") as pp_c,
            tc.tile_pool(name="dram", bufs=1, space="DRAM") as dram,
        ):
            # Dummy collective issued as the very first thing: the first
            # collective in a NEFF absorbs the cross-core launch skew
            # (measured ~16us), so pay it here, overlapped with the x/W DMAs
            # and transposes, instead of inside the first real AllReduce.
            dcc_s = persist.tile([1, 8], F32, tag="dcc_s")
            nc.vector.memset(dcc_s, 0.0)
            dcc_i = dram.tile([1, 8], F32, tag="dcc_i")
            dcc_o = dram.tile([1, 8], F32, tag="dcc_o")
            nc.gpsimd.dma_start(out=dcc_i[:, :], in_=dcc_s[:, :])
            nc.gpsimd.collective_compute(
                "AllReduce",
                mybir.AluOpType.add,
                replica_groups=[list(range(NCORES))],
                ins=[dcc_i[:, :]],
                outs=[dcc_o[:, :]],
            )

            # ---- constants ----
            # id64 twice: transposes of x blocks living at partition base 0
            # (i-tiles 0..3) and base 64 (i-tiles 4..8) need a same-base rhs.
            id64 = persist.tile([128, 64], dt_in, tag="id64")
            make_identity(nc, id64[0:64, :])
            nc.sync.dma_start(out=id64[64:128, :], in_=id64[0:64, :])
            id128 = persist.tile([128, 128], F32, tag="id128")
            make_identity(nc, id128)
            eps_c = persist.tile([64, 1], F32, tag="eps_c")
            nc.vector.memset(eps_c, 1e-8)
            ones1 = persist.tile([1, 64], F32, tag="ones1")
            nc.vector.memset(ones1, 1.0)

            # ---- load x (four concurrent DMAs, two per partition half) ----
            xw = persist.tile([128, 5120], dt_in, tag="xw")
            nc.sync.dma_start(out=xw[0:64, 0:2048], in_=x_d[:, 0:2048])
            nc.sync.dma_start(out=xw[0:64, 2048:4096], in_=x_d[:, 2048:4096])
            nc.sync.dma_start(out=xw[64:128, 0:2560], in_=x_d[:, 4096:6656])
            nc.sync.dma_start(out=xw[64:128, 2560:5120], in_=x_d[:, 6656:9216])

            # ---- load W (9 tiles; per-tile DMA for queue parallelism) ----
            w_sb = persist.tile([128, NT, Q * O * P], dt_in, tag="w_sb")
            w_rows = w_d.rearrange("(t p) f -> t p f", p=128)
            for t in range(NT):
                nc.sync.dma_start(out=w_sb[:, t, :], in_=w_rows[t])



            # ---- transpose x into xT[(t,q) blocks of [128(i), 64(b)]] ----
            xT = persist.tile([128, NT * Q * 64], dt_in, tag="xT")
            for t in range(NT):
                for q in range(Q):
                    pt = pp_small.tile([128, 64], dt_in, tag="pt", name="pt")
                    ident = id64[0:64, :] if t < 4 else id64[64:128, :]
                    nc.tensor.transpose(pt, _x_block(xw, t, q), ident)
                    # split PSUM->SBUF copies between ACT and DVE
                    if (t * Q + q) % 2 == 0:
                        nc.scalar.copy(xT[:, ts(t * Q + q, 64)], pt)
                    else:
                        nc.vector.tensor_copy(xT[:, ts(t * Q + q, 64)], pt)

            # ---- persistent routing state ----
            bT = persist.tile([10, I], F32, tag="bT")          # logits, [o, i]
            u_acc = persist.tile([128, NT, O], F32, tag="u_acc")
            A = persist.tile([128, NT, Q, O, P], dt_in, tag="A")

            # fp16 collective payload: ~9us of the ~13-18us AllReduce cost
            # scales with size, and fp16 logits cost only ~3e-4 relative.
            cc_in = dram.tile([10, I], F16, tag="cc_in")
            cc_out = dram.tile([10, I], F16, tag="cc_out")

            for k in range(NUM_ITERS):
                # ---- c = softmax(b) over i (skip: iter 0 is uniform) ----
                if k > 0:
                    negmax = scratch.tile([10, 1], F32, tag="negmax", name="negmax")
                    nc.vector.tensor_reduce(
                        negmax, bT, axis=mybir.AxisListType.X,
                        op=mybir.AluOpType.max, negate=True,
                    )
                    eT = scratch.tile([10, I], F32, tag="eT", name="eT")
                    esum = scratch.tile([10, 1], F32, tag="esum", name="esum")
                    nc.scalar.activation(
                        out=eT, in_=bT, func=mybir.ActivationFunctionType.Exp,
                        bias=negmax, scale=1.0, accum_out=esum,
                    )
                    nc.vector.reciprocal(esum, esum)
                    cT = scratch.tile([10, I], F32, tag="cT", name="cT")
                    nc.vector.tensor_scalar_mul(cT, eT, esum)
                    # per i-tile: transpose c back to [i, o] (fp16), then
                    # A = c * W broadcast-mult split across DVE and GpSimd.
                    for t in range(NT):
                        pc = pp_small.tile([128, 10], F32, tag="pt", name="pc")
                        nc.tensor.transpose(
                            pc, cT[:, ts(t, 128)], id128[0:10, 0:10]
                        )
                        c16n = scratch.tile(
                            [128, O], dt_in, tag="c16n", name="c16n", bufs=3
                        )
                        nc.scalar.copy(c16n, pc)
                        # split the broadcast mult across DVE and GpSimd so
                        # neither engine gates the s-matmuls behind it; the
                        # slower GpSimd (2.9us/tile vs DVE 1.4us) takes only
                        # the last three tiles.
                        eng = nc.vector if t < 6 else nc.gpsimd
                        eng.tensor_mul(
                            A[:, t],
                            _w_qop(w_sb, t),
                            c16n[:, None, :, None].broadcast_to(
                                [128, Q, O, P]
                            ),
                        )

                # ---- s = sum_{i,q} A[iq, op] x[b, iq]  (72 matmuls, one PSUM) ----
                ps = pp_s.tile([64, OP], F32, tag="ps", name="ps")
                first, last = (0, 0), (NT - 1, Q - 1)
                for t in range(NT):
                    for q in range(Q):
                        rhs = (
                            _w_qop(w_sb, t)[:, q] if k == 0 else A[:, t, q]
                        )
                        nc.tensor.matmul(
                            ps,
                            lhsT=xT[:, ts(t * Q + q, 64)],
                            rhs=rhs,
                            start=(t, q) == first,
                            stop=(t, q) == last,
                        )
                s_sb = scratch.tile([64, O, P], F32, tag="s_sb", name="s_sb")
                nc.scalar.mul(
                    s_sb.rearrange("b o p -> b (o p)"), ps,
                    1.0 / I if k == 0 else 1.0,
                )

                # ---- v = squash(s) ----
                sqs = scratch.tile([64, O, P], F32, tag="sqs", name="sqs")
                nc.vector.tensor_mul(sqs, s_sb, s_sb)
                sq = scratch.tile([64, O], F32, tag="sq", name="sq")
                nc.vector.reduce_sum(sq, sqs, axis=mybir.AxisListType.X)
                t1 = scratch.tile([64, O], F32, tag="t1", name="t1")
                nc.scalar.activation(
                    out=t1, in_=sq, func=mybir.ActivationFunctionType.Sqrt,
                    bias=eps_c, scale=1.0,
                )
                t2 = scratch.tile([64, O], F32, tag="t2", name="t2")
                # t2 = (sq + 1) * sqrt(sq + eps), fused in one DVE op
                nc.vector.scalar_tensor_tensor(
                    out=t2, in0=sq, scalar=1.0, in1=t1,
                    op0=mybir.AluOpType.add, op1=mybir.AluOpType.mult,
                )
                nc.vector.reciprocal(t2, t2)
                nc.vector.tensor_mul(t2, sq, t2)   # t2 = sq/((1+sq)*sqrt(sq+eps))
                v_sb = scratch.tile([64, O, P], F32, tag="v_sb", name="v_sb")
                nc.vector.tensor_mul(
                    v_sb, s_sb, t2[:, :, None].broadcast_to([64, O, P])
                )

                if k == NUM_ITERS - 1:
                    nc.sync.dma_start(
                        out=v_d[:, :],
                        in_=v_sb.rearrange("b o p -> b (o p)"),
                    )
                    continue

                # fp16 copies of v (partition bases 0 and 64) for the
                # agreement matmul rhs, matching each x block's base.
                v16 = scratch.tile([128, OP], dt_in, tag="v16", name="v16")
                vf = v_sb.rearrange("b o p -> b (o p)")
                nc.scalar.copy(v16[0:64, :], vf)
                nc.scalar.copy(v16[64:128, :], vf)

                # ---- agreement: u[i,o] = sum_{b,p,q} W x v ----
                # C lands in PSUM fp32; ACT copies it to fp16 SBUF so the
                # DVE mult runs in the 16-bit 2x perf mode. The q-reduction
                # is a 2x fp16 add-tree; only the final p-reduction runs at
                # the DVE's 1x reduce rate.
                uT = scratch.tile([10, I], dt_in, tag="uT", name="uT")
                for t in range(NT):
                    prods = []
                    for h in range(2):       # q in quads
                        pc4 = pp_c.tile([128, 1024], F32, tag="pc4", name="pc4")
                        for qq in range(4):
                            q = h * 4 + qq
                            nc.tensor.matmul(
                                pc4[:, qq * 256 : qq * 256 + OP],
                                lhsT=_x_block(xw, t, q),
                                rhs=v16[0:64] if t < 4 else v16[64:128],
                                start=True,
                                stop=True,
                            )
                        cview = pc4.rearrange("p (q f) -> p q f", q=4)[
                            :, :, 0:OP
                        ].rearrange("p q (o pp) -> p q o pp", o=O)
                        c16 = scratch.tile(
                            [128, 4, O, P], dt_in, tag="c16", name="c16",
                            bufs=3,
                        )
                        nc.scalar.copy(c16, cview)
                        prod = scratch.tile(
                            [128, 4, O, P], dt_in, tag=f"prod{h}",
                            name="prod", bufs=2,
                        )
                        nc.vector.tensor_mul(
                            prod, c16, _w_qop(w_sb, t)[:, ts(h, 4)]
                        )
                        prods.append(prod)
                    s4 = scratch.tile(
                        [128, 4, O, P], dt_in, tag="s4", name="s4"
                    )
                    nc.vector.tensor_add(s4, prods[0], prods[1])
                    s2 = scratch.tile(
                        [128, 2, O, P], dt_in, tag="s2", name="s2"
                    )
                    nc.vector.tensor_add(s2, s4[:, 0:2], s4[:, 2:4])
                    s1 = scratch.tile([128, O, P], dt_in, tag="s1", name="s1")
                    nc.vector.tensor_add(s1, s2[:, 0], s2[:, 1])
                    nc.vector.tensor_reduce(
                        u_acc[:, t, :], s1, axis=mybir.AxisListType.X,
                        op=mybir.AluOpType.add,
                    )
                # ---- transpose u -> [o, i], AllReduce, update b ----
                for t in range(NT):
                    pu = pp_small.tile([10, 128], F32, tag="pt", name="pu")
                    nc.tensor.transpose(pu, u_acc[:, t, :], id128)
                    nc.scalar.copy(uT[:, ts(t, 128)], pu)
                nc.gpsimd.dma_start(out=cc_in[:], in_=uT[:])
                nc.gpsimd.collective_compute(
                    "AllReduce",
                    mybir.AluOpType.add,
                    replica_groups=[list(range(NCORES))],
                    ins=[cc_in[:]],
                    outs=[cc_out[:]],
                )
                # Filler matmuls: the AllReduce stalls the PE ~12-20us and
                # the HAM clock-gate then holds the PE at reduced speed for
                # the next phase. Keep it busy with throwaway matmuls during
                # the collective wait; reading uT makes them start only once
                # the collective inputs are ready (not earlier).
                # the first AllReduce runs ~19us, the second ~12us
                for f in range(60 if k == 0 else 36):
                    pf = pp_s.tile([64, OP], F32, tag="pf", name="pf")
                    nc.tensor.matmul(
                        pf,
                        lhsT=id64[0:10, :],
                        rhs=uT[:, ts(f % 7, OP)],
                        start=True,
                        stop=True,
                    )
                uTr = scratch.tile([10, I], dt_in, tag="uTr", name="uTr")
                nc.gpsimd.dma_start(out=uTr[:], in_=cc_out[:])
                if k == 0:
                    nc.vector.tensor_scalar_mul(bT, uTr, 1.0 / B)
                else:
                    nc.vector.scalar_tensor_tensor(
                        out=bT, in0=uTr, scalar=1.0 / B, in1=bT,
                        op0=mybir.AluOpType.mult, op1=mybir.AluOpType.add,
                    )

    nc.compile()
    return nc


_CACHE = {}


def _get_nc():
    if "nc" not in _CACHE:
        _CACHE["nc"] = build_digitcaps()
    return _CACHE["nc"]


def _prep_w(W: np.ndarray) -> np.ndarray:
    # [i, o, p, q] -> [i, (q, o, p)] so on-chip reads are contiguous
    return np.ascontiguousarray(
        W.transpose(0, 3, 1, 2).astype(NP_IN).reshape(I, Q * O * P)
    )


def kernel(x: np.ndarray, W: np.ndarray) -> np.ndarray:
    assert x.shape == (B, I, Q) and W.shape == (I, O, P, Q)
    nc = _get_nc()
    xf = np.ascontiguousarray(x.reshape(B, IQ).astype(NP_IN))
    wf = _prep_w(W)
    in_maps = [
        {"x": xf[c * BL : (c + 1) * BL], "w": wf} for c in range(NCORES)
    ]
    res = run_bass_kernel_spmd(nc, in_maps, list(range(NCORES)))
    out = np.concatenate([res.results[c]["v"] for c in range(NCORES)], axis=0)
    return out.reshape(B, O, P)
